# revision 21
# baseline (speedup 1.0000x reference)
"""Multi-head attention (B=2, L=2048, D=1024, H=16) on 8 Trainium2 NeuronCores.

Sharding: tensor-parallel over heads - 2 heads per core. Each core:
  - computes Q/K head projections transposed (QHT/KHT: [128=2*64 d, L] per
    batch), V projection transposed (VHT, scratch aliased onto the context
    buffer) then PE-transposed to natural VH [j, d] with a ones column
    appended so the attention-weighted-sum matmul also yields the softmax
    denominators for free;
  - scores are computed transposed (S^T: keys on partitions, queries on the
    free axis, 1024-wide blocks so attention stores are 4KB-contiguous),
    exp'd on the scalar engine, normalized with a broadcast reciprocal
    (bounced through DRAM to reach all partitions), and written to HBM
    transposed ([bh, j, i]);
  - the output projection uses the core's 128 context columns against the
    matching rows of w_o, yielding a partial [4096, 1024] output.
Host glue: transposes inputs/weights, sums the 8 partial outputs, and
re-transposes the attention tiles into [B, H, L, L].

Matmuls run as float32r (full PE rate; ~1.5e-4 rel err measured on HW).
"""

import os
import concurrent.futures as _fut

import numpy as np

import concourse.bacc as bacc
import concourse.mybir as mybir
import concourse.tile as tile
from concourse.bass_utils import run_bass_kernel_spmd

B = 2
L = 2048
D = 1024
H = 16
DH = 64
NCORES = 8
HPC = H // NCORES          # heads per core
CW = HPC * DH              # per-core head width (128)
R = B * L                  # 4096 token rows
P = 128

FP32 = mybir.dt.float32
FP32R = mybir.dt.float32r
AF = mybir.ActivationFunctionType

IT = 512                   # i-tile width (queries per block)
NI = L // IT               # i tiles per batch (4)
NJ = L // P                # j tiles per batch (16)
NF = D // P                # contraction tiles (8)
HIT = 512                  # fp32r moving-operand cap

_CACHE = {}


def _build():
    nc = bacc.Bacc("TRN2")

    qt = nc.dram_tensor("qt", [D, R], FP32R, kind="ExternalInput")
    kt = nc.dram_tensor("kt", [D, R], FP32R, kind="ExternalInput")
    vt = nc.dram_tensor("vt", [D, R], FP32R, kind="ExternalInput")
    wqt = nc.dram_tensor("wqt", [D, CW], FP32R, kind="ExternalInput")
    wkt = nc.dram_tensor("wkt", [D, CW], FP32R, kind="ExternalInput")
    wvt = nc.dram_tensor("wvt", [D, CW], FP32R, kind="ExternalInput")
    wot = nc.dram_tensor("wot", [CW, D], FP32R, kind="ExternalInput")
    bq8 = nc.dram_tensor("bq8", [CW, 1], FP32, kind="ExternalInput")
    bk = nc.dram_tensor("bk", [CW, 1], FP32, kind="ExternalInput")
    bv = nc.dram_tensor("bv", [CW, 1], FP32, kind="ExternalInput")
    bo = nc.dram_tensor("bo", [1, D], FP32, kind="ExternalInput")
    iden = nc.dram_tensor("iden", [P, P], FP32R, kind="ExternalInput")

    rscr = nc.dram_tensor("rscr", [B * HPC * NI, IT], FP32)
    attn_t = nc.dram_tensor("attn_t", [B * HPC, L, L], FP32, kind="ExternalOutput")
    outp = nc.dram_tensor("outp", [R, D], FP32, kind="ExternalOutput")

    with tile.TileContext(nc) as tc:
        with (
            tc.tile_pool(name="const", bufs=1) as cpool,
            tc.tile_pool(name="resid", bufs=1) as rpool,
            tc.tile_pool(name="ps_s", bufs=3, space="PSUM") as ps_s,
            tc.tile_pool(name="ps_ctx", bufs=2, space="PSUM") as ps_ctx,
            tc.tile_pool(name="stage", bufs=5) as stage,
            tc.tile_pool(name="expp", bufs=20) as epool,
            tc.tile_pool(name="small", bufs=2) as spool,
            tc.tile_pool(name="small1", bufs=1) as spool1,
            tc.tile_pool(name="osbp", bufs=3) as osbp,
        ):
            # ---- constants ----
            wq_sb = cpool.tile([P, NF, CW], FP32R, tag="wq")
            wk_sb = cpool.tile([P, NF, CW], FP32R, tag="wk")
            wv_sb = cpool.tile([P, NF, CW], FP32R, tag="wv")
            wo_sb = cpool.tile([CW, D], FP32R, tag="wo")
            nc.sync.dma_start(wq_sb[:], wqt.rearrange("(fo p) m -> p fo m", p=P))
            nc.sync.dma_start(wk_sb[:], wkt.rearrange("(fo p) m -> p fo m", p=P))
            nc.sync.dma_start(wv_sb[:], wvt.rearrange("(fo p) m -> p fo m", p=P))
            nc.sync.dma_start(wo_sb[:], wot[:])
            bq_sb = cpool.tile([CW, 1], FP32, tag="bq")
            bk_sb = cpool.tile([CW, 1], FP32, tag="bk")
            bv_sb = cpool.tile([CW, 1], FP32, tag="bv")
            nc.sync.dma_start(bq_sb[:], bq8[:])
            nc.sync.dma_start(bk_sb[:], bk[:])
            nc.sync.dma_start(bv_sb[:], bv[:])
            ident = cpool.tile([P, P], FP32R, tag="ident")
            nc.sync.dma_start(ident[:], iden[:])
            ones1 = cpool.tile([1, P], FP32, tag="ones1")
            nc.gpsimd.memset(ones1[:], 1.0)
            ones_col = cpool.tile([P, 1], FP32, tag="ones_col")
            nc.gpsimd.memset(ones_col[:], 1.0)
            bo_bc = cpool.tile([P, D], FP32, tag="bobc")
            nc.sync.dma_start(bo_bc[:], bo[:].to_broadcast((P, D)))

            # ---- residents (per batch) ----
            qht = [rpool.tile([P, L], FP32R, tag=f"qht{b}", name=f"qht{b}")
                   for b in range(B)]
            kht = [rpool.tile([P, L], FP32R, tag=f"kht{b}", name=f"kht{b}")
                   for b in range(B)]
            vh = [rpool.tile([P, NJ, HPC, DH + 1], FP32R, tag=f"vh{b}",
                             name=f"vh{b}") for b in range(B)]
            ctxt = [rpool.tile([P, L], FP32R, tag=f"ctxt{b}", name=f"ctxt{b}")
                    for b in range(B)]
            vht = ctxt  # scratch alias: VHT dies before ctxt is written
            for b in range(B):
                nc.vector.tensor_copy(
                    vh[b][:, :, :, DH:DH + 1],
                    ones_col[:, None, None, :].to_broadcast((P, NJ, HPC, 1)),
                )

            # projections (transposed): XHT[d, i] = sum_f w_x[d, f] xt[f, i]
            def project_block(xt_dram, w_sb, dst, bias_sb, scale, b, i8):
                ps = ps_s.tile([P, IT], FP32, tag="s")
                for f2 in range(NF // 2):
                    st = stage.tile([P, 2, IT], FP32R, tag="stage")
                    nc.sync.dma_start(
                        st[:],
                        xt_dram[
                            2 * f2 * P:(2 * f2 + 2) * P,
                            b * L + i8 * IT:b * L + (i8 + 1) * IT,
                        ].rearrange("(t p) i -> p t i", p=P),
                    )
                    for t in range(2):
                        f = 2 * f2 + t
                        for h in range(IT // HIT):
                            nc.tensor.matmul(
                                ps[:, h * HIT:(h + 1) * HIT],
                                w_sb[:, f, :], st[:, t, h * HIT:(h + 1) * HIT],
                                start=(f == 0), stop=(f == NF - 1),
                            )
                nc.scalar.activation(
                    dst[:, i8 * IT:(i8 + 1) * IT], ps[:],
                    AF.Identity, bias=bias_sb[:], scale=scale,
                )

            def vh_transpose(b, jt):
                pst = ps_s.tile([P, P], FP32R, tag="s")
                nc.tensor.transpose(
                    pst[:], vht[b][:, jt * P:(jt + 1) * P], ident[:]
                )
                for hl in range(HPC):
                    nc.vector.tensor_copy(
                        vh[b][:, jt, hl, 0:DH], pst[:, hl * DH:(hl + 1) * DH]
                    )

            def project_all(b):
                for w_sb_, dst_, bias_, scale_, src_ in (
                    (wq_sb, qht[b], bq_sb, 0.125, qt),
                    (wk_sb, kht[b], bk_sb, 1.0, kt),
                    (wv_sb, vht[b], bv_sb, 1.0, vt),
                ):
                    for i8 in range(L // IT):
                        project_block(src_, w_sb_, dst_, bias_, scale_, b, i8)
                for jt in range(NJ):
                    vh_transpose(b, jt)

            def attention(b):
                for hl in range(HPC):
                    dsl = slice(hl * DH, (hl + 1) * DH)
                    pidx = b * HPC + hl
                    for i4 in range(NI):
                        io = i4 * IT
                        psc = ps_ctx.tile([P, IT], FP32, tag="ctx")
                        ets = []
                        for tp in range(NJ // 2):
                            et = epool.tile([P, 2 * IT], FP32R, tag="e")
                            ets.append(et)
                            pss = ps_s.tile([P, 2 * IT], FP32, tag="s")
                            for h2 in range(2):
                                jt = 2 * tp + h2
                                nc.tensor.matmul(
                                    pss[:, h2 * IT:(h2 + 1) * IT],
                                    kht[b][dsl, jt * P:(jt + 1) * P],
                                    qht[b][dsl, io:io + IT],
                                    start=True, stop=True,
                                )
                            nc.scalar.activation(et[:], pss[:], AF.Exp)
                            for h2 in range(2):
                                jt = 2 * tp + h2
                                nc.tensor.matmul(
                                    psc[0:DH + 1, :],
                                    vh[b][:, jt, hl, :],
                                    et[:, h2 * IT:(h2 + 1) * IT],
                                    start=(jt == 0), stop=(jt == NJ - 1),
                                )
                        recip = spool1.tile([1, IT], FP32, tag="recip")
                        nc.vector.reciprocal(recip[:], psc[DH:DH + 1, :])
                        psb = ps_s.tile([P, IT], FP32, tag="s")
                        nc.tensor.matmul(
                            psb[:], ones1[:], recip[:], start=True, stop=True
                        )
                        bc = spool.tile([P, IT], FP32, tag="bc")
                        nc.vector.tensor_copy(bc[:], psb[:])
                        # context normalize -> CTXT
                        nc.vector.tensor_mul(
                            ctxt[b][dsl, io:io + IT], psc[0:DH, :], bc[0:DH, :]
                        )
                        # attention normalize (in place) + batched stores
                        for tp in range(NJ // 2):
                            et = ets[tp]
                            eng = nc.vector if tp % 8 in (0, 3, 6) else nc.gpsimd
                            eng.tensor_mul(
                                et[:].rearrange("p (t i) -> p t i", t=2),
                                et[:].rearrange("p (t i) -> p t i", t=2),
                                bc[:, None, :].to_broadcast((P, 2, IT)),
                            )
                            nc.sync.dma_start(
                                attn_t[
                                    pidx,
                                    2 * tp * P:(2 * tp + 2) * P,
                                    i4 * IT:(i4 + 1) * IT,
                                ].rearrange("(t p) i -> p t i", p=P),
                                et[:].rearrange("p (t i) -> p t i", t=2)
                                .bitcast(FP32),
                            )

            def out_proj(b):
                for it in range(L // P):
                    osb = osbp.tile([P, D], FP32, tag="osb")
                    pso = ps_s.tile([P, D], FP32, tag="s")
                    for nh in range(D // HIT):
                        nc.tensor.matmul(
                            pso[:, nh * HIT:(nh + 1) * HIT],
                            ctxt[b][:, it * P:(it + 1) * P],
                            wo_sb[:, nh * HIT:(nh + 1) * HIT],
                            start=True, stop=True,
                        )
                    nc.vector.tensor_add(osb[:], pso[:], bo_bc[:])
                    nc.sync.dma_start(
                        outp[b * L + it * P:b * L + (it + 1) * P, :], osb[:]
                    )

            project_all(0)
            project_all(1)
            attention(0)
            out_proj(0)
            attention(1)
            out_proj(1)

    nc.compile()
    return nc


def _get_nc():
    if "nc" not in _CACHE:
        _CACHE["nc"] = _build()
    return _CACHE["nc"]


def kernel(q, k, v, w_q, b_q, w_k, b_k, w_v, b_v, w_o, b_o):
    q = np.asarray(q, dtype=np.float32)
    k = np.asarray(k, dtype=np.float32)
    v = np.asarray(v, dtype=np.float32)
    w_q = np.asarray(w_q, dtype=np.float32)
    w_k = np.asarray(w_k, dtype=np.float32)
    w_v = np.asarray(w_v, dtype=np.float32)
    w_o = np.asarray(w_o, dtype=np.float32)
    b_q = np.asarray(b_q, dtype=np.float32)
    b_k = np.asarray(b_k, dtype=np.float32)
    b_v = np.asarray(b_v, dtype=np.float32)
    b_o = np.asarray(b_o, dtype=np.float32)

    qt = np.ascontiguousarray(q.reshape(R, D).T)
    kt = np.ascontiguousarray(k.reshape(R, D).T)
    vt = np.ascontiguousarray(v.reshape(R, D).T)

    in_maps = []
    for c in range(NCORES):
        sl = slice(c * CW, (c + 1) * CW)
        in_maps.append({
            "iden": np.eye(P, dtype=np.float32),
            "qt": qt,
            "kt": kt,
            "vt": vt,
            "wqt": np.ascontiguousarray(w_q[sl, :].T),
            "wkt": np.ascontiguousarray(w_k[sl, :].T),
            "wvt": np.ascontiguousarray(w_v[sl, :].T),
            "wot": np.ascontiguousarray(w_o[:, sl].T),
            "bq8": np.ascontiguousarray((b_q[sl] / 8.0).reshape(CW, 1)),
            "bk": np.ascontiguousarray(b_k[sl].reshape(CW, 1)),
            "bv": np.ascontiguousarray(b_v[sl].reshape(CW, 1)),
            "bo": np.ascontiguousarray(
                (b_o if c == 0 else np.zeros_like(b_o)).reshape(1, D)
            ),
        })

    nc = _get_nc()

    prof_dir = os.environ.get("KERNEL_PROF_DIR")
    if prof_dir:
        from trn_agent_boot.trn_boot import _ntff_profile_via_ctypes

        hook = _ntff_profile_via_ctypes("/opt/axon/libaxon_pjrt.so")
        with hook(prof_dir, [0]):
            res = run_bass_kernel_spmd(nc, in_maps, list(range(NCORES)))
    else:
        res = run_bass_kernel_spmd(nc, in_maps, list(range(NCORES)))

    out = res.results[0]["outp"].copy()
    for c in range(1, NCORES):
        out += res.results[c]["outp"]
    out = out.reshape(B, L, D)

    attn = np.empty((B, H, L, L), dtype=np.float32)

    def _fill(c):
        at = res.results[c]["attn_t"]
        for b in range(B):
            for hl in range(HPC):
                attn[b, c * HPC + hl] = at[b * HPC + hl].T

    with _fut.ThreadPoolExecutor(max_workers=8) as ex:
        list(ex.map(_fill, range(NCORES)))

    return out, attn


# revision 22
# speedup vs baseline: 1.1484x; 1.1484x over previous
"""Multi-head attention (B=2, L=2048, D=1024, H=16) on 8 Trainium2 NeuronCores.

Sharding: tensor-parallel over heads - 2 heads per core. Each core:
  - computes Q/K head projections transposed (QHT/KHT: [128=2*64 d, L] per
    batch), V projection transposed (VHT, scratch aliased onto the context
    buffer) then PE-transposed to natural VH [j, d] with a ones column
    appended so the attention-weighted-sum matmul also yields the softmax
    denominators for free;
  - scores are computed transposed (S^T: keys on partitions, queries on the
    free axis, 1024-wide blocks so attention stores are 4KB-contiguous),
    exp'd on the scalar engine, normalized with a broadcast reciprocal
    (bounced through DRAM to reach all partitions), and written to HBM
    transposed ([bh, j, i]);
  - the output projection uses the core's 128 context columns against the
    matching rows of w_o, yielding a partial [4096, 1024] output.
Host glue: transposes inputs/weights, sums the 8 partial outputs, and
re-transposes the attention tiles into [B, H, L, L].

Matmuls run as float32r (full PE rate; ~1.5e-4 rel err measured on HW).
"""

import os
import concurrent.futures as _fut

import numpy as np

import concourse.bacc as bacc
import concourse.mybir as mybir
import concourse.tile as tile
from concourse.bass_utils import run_bass_kernel_spmd

B = 2
L = 2048
D = 1024
H = 16
DH = 64
NCORES = 8
HPC = H // NCORES          # heads per core
CW = HPC * DH              # per-core head width (128)
R = B * L                  # 4096 token rows
P = 128

FP32 = mybir.dt.float32
FP32R = mybir.dt.float32r
AF = mybir.ActivationFunctionType

IT = 512                   # i-tile width (queries per block)
NI = L // IT               # i tiles per batch (4)
NJ = L // P                # j tiles per batch (16)
NF = D // P                # contraction tiles (8)
HIT = 512                  # fp32r moving-operand cap

_CACHE = {}


def _build():
    nc = bacc.Bacc("TRN2")

    qt = nc.dram_tensor("qt", [D, R], FP32R, kind="ExternalInput")
    kt = nc.dram_tensor("kt", [D, R], FP32R, kind="ExternalInput")
    vt = nc.dram_tensor("vt", [D, R], FP32R, kind="ExternalInput")
    wqt = nc.dram_tensor("wqt", [D, CW], FP32R, kind="ExternalInput")
    wkt = nc.dram_tensor("wkt", [D, CW], FP32R, kind="ExternalInput")
    wvt = nc.dram_tensor("wvt", [D, CW], FP32R, kind="ExternalInput")
    wot = nc.dram_tensor("wot", [CW, D], FP32R, kind="ExternalInput")
    bq8 = nc.dram_tensor("bq8", [CW, 1], FP32, kind="ExternalInput")
    bk = nc.dram_tensor("bk", [CW, 1], FP32, kind="ExternalInput")
    bv = nc.dram_tensor("bv", [CW, 1], FP32, kind="ExternalInput")
    bo = nc.dram_tensor("bo", [1, D], FP32, kind="ExternalInput")
    iden = nc.dram_tensor("iden", [P, P], FP32R, kind="ExternalInput")

    rscr = nc.dram_tensor("rscr", [B * HPC * NI, IT], FP32)
    attn_t = nc.dram_tensor("attn_t", [B * HPC, L, L], FP32, kind="ExternalOutput")
    outp = nc.dram_tensor("outp", [R, D], FP32, kind="ExternalOutput")

    with tile.TileContext(nc) as tc:
        with (
            tc.tile_pool(name="const", bufs=1) as cpool,
            tc.tile_pool(name="resid", bufs=1) as rpool,
            tc.tile_pool(name="ps_s", bufs=3, space="PSUM") as ps_s,
            tc.tile_pool(name="ps_ctx", bufs=2, space="PSUM") as ps_ctx,
            tc.tile_pool(name="stage", bufs=5) as stage,
            tc.tile_pool(name="expp", bufs=20) as epool,
            tc.tile_pool(name="small", bufs=2) as spool,
            tc.tile_pool(name="small1", bufs=1) as spool1,
            tc.tile_pool(name="osbp", bufs=3) as osbp,
        ):
            # ---- constants ----
            wq_sb = cpool.tile([P, NF, CW], FP32R, tag="wq")
            wk_sb = cpool.tile([P, NF, CW], FP32R, tag="wk")
            wv_sb = cpool.tile([P, NF, CW], FP32R, tag="wv")
            wo_sb = cpool.tile([CW, D], FP32R, tag="wo")
            nc.sync.dma_start(wq_sb[:], wqt.rearrange("(fo p) m -> p fo m", p=P))
            nc.sync.dma_start(wk_sb[:], wkt.rearrange("(fo p) m -> p fo m", p=P))
            nc.sync.dma_start(wv_sb[:], wvt.rearrange("(fo p) m -> p fo m", p=P))
            nc.sync.dma_start(wo_sb[:], wot[:])
            bq_sb = cpool.tile([CW, 1], FP32, tag="bq")
            bk_sb = cpool.tile([CW, 1], FP32, tag="bk")
            bv_sb = cpool.tile([CW, 1], FP32, tag="bv")
            nc.sync.dma_start(bq_sb[:], bq8[:])
            nc.sync.dma_start(bk_sb[:], bk[:])
            nc.sync.dma_start(bv_sb[:], bv[:])
            ident = cpool.tile([P, P], FP32R, tag="ident")
            nc.sync.dma_start(ident[:], iden[:])
            ones1 = cpool.tile([1, P], FP32, tag="ones1")
            nc.gpsimd.memset(ones1[:], 1.0)
            ones_col = cpool.tile([P, 1], FP32, tag="ones_col")
            nc.gpsimd.memset(ones_col[:], 1.0)
            bo_bc = cpool.tile([P, D], FP32, tag="bobc")
            nc.sync.dma_start(bo_bc[:], bo[:].to_broadcast((P, D)))

            # ---- residents (per batch) ----
            qht = [rpool.tile([P, L], FP32R, tag=f"qht{b}", name=f"qht{b}")
                   for b in range(B)]
            kht = [rpool.tile([P, L], FP32R, tag=f"kht{b}", name=f"kht{b}")
                   for b in range(B)]
            vh = [rpool.tile([P, NJ, HPC, DH + 1], FP32R, tag=f"vh{b}",
                             name=f"vh{b}") for b in range(B)]
            ctxt = [rpool.tile([P, L], FP32R, tag=f"ctxt{b}", name=f"ctxt{b}")
                    for b in range(B)]
            vht = ctxt  # scratch alias: VHT dies before ctxt is written
            for b in range(B):
                nc.vector.tensor_copy(
                    vh[b][:, :, :, DH:DH + 1],
                    ones_col[:, None, None, :].to_broadcast((P, NJ, HPC, 1)),
                )

            # projections (transposed): XHT[d, i] = sum_f w_x[d, f] xt[f, i]
            def project_block(xt_dram, w_sb, dst, bias_sb, scale, b, i8):
                ps = ps_s.tile([P, IT], FP32, tag="s")
                for f2 in range(NF // 2):
                    st = stage.tile([P, 2, IT], FP32R, tag="stage")
                    nc.sync.dma_start(
                        st[:],
                        xt_dram[
                            2 * f2 * P:(2 * f2 + 2) * P,
                            b * L + i8 * IT:b * L + (i8 + 1) * IT,
                        ].rearrange("(t p) i -> p t i", p=P),
                    )
                    for t in range(2):
                        f = 2 * f2 + t
                        for h in range(IT // HIT):
                            nc.tensor.matmul(
                                ps[:, h * HIT:(h + 1) * HIT],
                                w_sb[:, f, :], st[:, t, h * HIT:(h + 1) * HIT],
                                start=(f == 0), stop=(f == NF - 1),
                            )
                nc.scalar.activation(
                    dst[:, i8 * IT:(i8 + 1) * IT], ps[:],
                    AF.Identity, bias=bias_sb[:], scale=scale,
                )

            def vh_transpose(b, jt):
                pst = ps_s.tile([P, P], FP32R, tag="s")
                nc.tensor.transpose(
                    pst[:], vht[b][:, jt * P:(jt + 1) * P], ident[:]
                )
                for hl in range(HPC):
                    nc.vector.tensor_copy(
                        vh[b][:, jt, hl, 0:DH], pst[:, hl * DH:(hl + 1) * DH]
                    )

            def project_all(b):
                for w_sb_, dst_, bias_, scale_, src_ in (
                    (wq_sb, qht[b], bq_sb, 0.125, qt),
                    (wk_sb, kht[b], bk_sb, 1.0, kt),
                    (wv_sb, vht[b], bv_sb, 1.0, vt),
                ):
                    for i8 in range(L // IT):
                        project_block(src_, w_sb_, dst_, bias_, scale_, b, i8)
                for jt in range(NJ):
                    vh_transpose(b, jt)

            def attention(b):
                for hl in range(HPC):
                    dsl = slice(hl * DH, (hl + 1) * DH)
                    pidx = b * HPC + hl
                    for i4 in range(NI):
                        io = i4 * IT
                        psc = ps_ctx.tile([P, IT], FP32, tag="ctx")
                        ets = []
                        for tp in range(NJ // 2):
                            et = epool.tile([P, 2 * IT], FP32R, tag="e")
                            ets.append(et)
                            pss = ps_s.tile([P, 2 * IT], FP32, tag="s")
                            for h2 in range(2):
                                jt = 2 * tp + h2
                                nc.tensor.matmul(
                                    pss[:, h2 * IT:(h2 + 1) * IT],
                                    kht[b][dsl, jt * P:(jt + 1) * P],
                                    qht[b][dsl, io:io + IT],
                                    start=True, stop=True,
                                )
                            nc.scalar.activation(et[:], pss[:], AF.Exp)
                            for h2 in range(2):
                                jt = 2 * tp + h2
                                nc.tensor.matmul(
                                    psc[0:DH + 1, :],
                                    vh[b][:, jt, hl, :],
                                    et[:, h2 * IT:(h2 + 1) * IT],
                                    start=(jt == 0), stop=(jt == NJ - 1),
                                )
                        recip = spool1.tile([1, IT], FP32, tag="recip")
                        nc.vector.reciprocal(recip[:], psc[DH:DH + 1, :])
                        psb = ps_s.tile([P, IT], FP32, tag="s")
                        nc.tensor.matmul(
                            psb[:], ones1[:], recip[:], start=True, stop=True
                        )
                        bc = spool.tile([P, IT], FP32, tag="bc")
                        nc.vector.tensor_copy(bc[:], psb[:])
                        # context normalize -> CTXT
                        nc.vector.tensor_mul(
                            ctxt[b][dsl, io:io + IT], psc[0:DH, :], bc[0:DH, :]
                        )
                        # attention normalize (in place) + batched stores
                        for tp in range(NJ // 2):
                            et = ets[tp]
                            eng = nc.vector if tp % 2 == 0 else nc.gpsimd
                            eng.tensor_mul(
                                et[:].rearrange("p (t i) -> p t i", t=2),
                                et[:].rearrange("p (t i) -> p t i", t=2),
                                bc[:, None, :].to_broadcast((P, 2, IT)),
                            )
                            nc.sync.dma_start(
                                attn_t[
                                    pidx,
                                    2 * tp * P:(2 * tp + 2) * P,
                                    i4 * IT:(i4 + 1) * IT,
                                ].rearrange("(t p) i -> p t i", p=P),
                                et[:].rearrange("p (t i) -> p t i", t=2)
                                .bitcast(FP32),
                            )

            def out_proj(b):
                for it in range(L // P):
                    osb = osbp.tile([P, D], FP32, tag="osb")
                    pso = ps_s.tile([P, D], FP32, tag="s")
                    for nh in range(D // HIT):
                        nc.tensor.matmul(
                            pso[:, nh * HIT:(nh + 1) * HIT],
                            ctxt[b][:, it * P:(it + 1) * P],
                            wo_sb[:, nh * HIT:(nh + 1) * HIT],
                            start=True, stop=True,
                        )
                    nc.vector.tensor_add(osb[:], pso[:], bo_bc[:])
                    nc.sync.dma_start(
                        outp[b * L + it * P:b * L + (it + 1) * P, :], osb[:]
                    )

            project_all(0)
            project_all(1)
            attention(0)
            out_proj(0)
            attention(1)
            out_proj(1)

    nc.compile()
    return nc


def _get_nc():
    if "nc" not in _CACHE:
        _CACHE["nc"] = _build()
    return _CACHE["nc"]


def kernel(q, k, v, w_q, b_q, w_k, b_k, w_v, b_v, w_o, b_o):
    q = np.asarray(q, dtype=np.float32)
    k = np.asarray(k, dtype=np.float32)
    v = np.asarray(v, dtype=np.float32)
    w_q = np.asarray(w_q, dtype=np.float32)
    w_k = np.asarray(w_k, dtype=np.float32)
    w_v = np.asarray(w_v, dtype=np.float32)
    w_o = np.asarray(w_o, dtype=np.float32)
    b_q = np.asarray(b_q, dtype=np.float32)
    b_k = np.asarray(b_k, dtype=np.float32)
    b_v = np.asarray(b_v, dtype=np.float32)
    b_o = np.asarray(b_o, dtype=np.float32)

    qt = np.ascontiguousarray(q.reshape(R, D).T)
    kt = np.ascontiguousarray(k.reshape(R, D).T)
    vt = np.ascontiguousarray(v.reshape(R, D).T)

    in_maps = []
    for c in range(NCORES):
        sl = slice(c * CW, (c + 1) * CW)
        in_maps.append({
            "iden": np.eye(P, dtype=np.float32),
            "qt": qt,
            "kt": kt,
            "vt": vt,
            "wqt": np.ascontiguousarray(w_q[sl, :].T),
            "wkt": np.ascontiguousarray(w_k[sl, :].T),
            "wvt": np.ascontiguousarray(w_v[sl, :].T),
            "wot": np.ascontiguousarray(w_o[:, sl].T),
            "bq8": np.ascontiguousarray((b_q[sl] / 8.0).reshape(CW, 1)),
            "bk": np.ascontiguousarray(b_k[sl].reshape(CW, 1)),
            "bv": np.ascontiguousarray(b_v[sl].reshape(CW, 1)),
            "bo": np.ascontiguousarray(
                (b_o if c == 0 else np.zeros_like(b_o)).reshape(1, D)
            ),
        })

    nc = _get_nc()

    prof_dir = os.environ.get("KERNEL_PROF_DIR")
    if prof_dir:
        from trn_agent_boot.trn_boot import _ntff_profile_via_ctypes

        hook = _ntff_profile_via_ctypes("/opt/axon/libaxon_pjrt.so")
        with hook(prof_dir, [0]):
            res = run_bass_kernel_spmd(nc, in_maps, list(range(NCORES)))
    else:
        res = run_bass_kernel_spmd(nc, in_maps, list(range(NCORES)))

    out = res.results[0]["outp"].copy()
    for c in range(1, NCORES):
        out += res.results[c]["outp"]
    out = out.reshape(B, L, D)

    attn = np.empty((B, H, L, L), dtype=np.float32)

    def _fill(c):
        at = res.results[c]["attn_t"]
        for b in range(B):
            for hl in range(HPC):
                attn[b, c * HPC + hl] = at[b * HPC + hl].T

    with _fut.ThreadPoolExecutor(max_workers=8) as ex:
        list(ex.map(_fill, range(NCORES)))

    return out, attn
